# revision 37
# baseline (speedup 1.0000x reference)
"""Complex multihead attention v3: fp16 PE, V-Karatsuba, transposed AV with
fused softmax sums, DMA transposes, qi-outer loop with inline O projection.

Sharding: data-parallel over batch (B=4) x tensor-parallel over heads
(16 heads -> 2 groups of 8). core = b*2 + head_group. Host combines partials.

Math notes (validated against reference):
 - K bias dropped (softmax invariant); V bias folded to host constant;
   Q bias applied at PSUM evacuation (per-partition ACT bias).
 - Q projection: stacked-real trick -> qcat = [qr; qi] per head.
 - K projection: same stacked psum [kr; ki], evacuated TWICE with
   cross-partition-offset activations: kcat1 = [kr; -ki], kcat2 = [ki; kr],
   so scores are st_r = kcat1^T qcat, st_i = kcat2^T qcat (no per-head
   query prep in the attention phase).
 - Q/K/V projections: Karatsuba M1=xr*A, M2=xi*B, M3=(xr+xi)*(A+B) with
   xs = xr+xi computed on-device (DVE) to cut x DMA traffic by 1/3.
 - Scores transposed [k, q]; exp on scalar engine (fp16 out).
 - AV transposed: out[q, f] = P[:,qc]^T @ [1|Vr|Vi]: softmax sums ride in
   col 0; normalizer = per-partition activation scale. Both r and i P parts
   multiply the SAME [1|Vr|Vi] tile (no [1|Vi|Vr] mirror): bankR = [s|PrVr|
   PrVi], bankI = [s|PiVr|PiVi]; combine crosses halves. attn [q,f] -> [f,q]
   via DMA xbar transpose (split in partition halves for latency).
 - O projection per qi-half inline (overlaps the other half's attention).
 - DMAs: consumption-ordered, section/half granularity; 40-buf x pool so
   next-phase x prefetch streams behind the current phase's matmuls.
"""

import numpy as np

import concourse.bass as bass
from concourse import bacc
import concourse.mybir as mybir
import concourse.tile as tile
from concourse.bass_utils import run_bass_kernel_spmd

S, B, E, H, D = 1024, 4, 1024, 16, 64
HPC = 8            # heads per core
EH = HPC * D       # 512
N_CORES = 8
F32 = mybir.dt.float32
F16 = mybir.dt.float16
AF = mybir.ActivationFunctionType

_NC_CACHE = []

VW = 130           # vext per-head stride ([1|Vr|Vi] = 129, +1 pad)


def _emit(tc):
    nc = tc.nc
    # x payload: rows 0:1024 = x_r^T, rows 1024:2048 = x_i^T (xs on-device)
    xq = nc.dram_tensor("xq", [2 * E, S], F16, kind="ExternalInput").ap()
    xk = nc.dram_tensor("xk", [2 * E, S], F16, kind="ExternalInput").ap()
    xv = nc.dram_tensor("xv", [2 * E, S], F16, kind="ExternalInput").ap()
    wq = nc.dram_tensor("wq", [E, 3 * EH], F16, kind="ExternalInput").ap()
    wk = nc.dram_tensor("wk", [E, 3 * EH], F16, kind="ExternalInput").ap()
    wv = nc.dram_tensor("wv", [E, 3 * EH], F16, kind="ExternalInput").ap()
    # O-proj weights pre-swizzled on host: [128, (part,m,j,n)] so each
    # (part,m) chunk is one contiguous 2KB-per-partition DMA.
    wot = nc.dram_tensor("wot", [128, 16 * 8 * 128], F16,
                         kind="ExternalInput").ap()
    # col rt: lanes 0:64 = -bq_r(head 2rt), 64:128 = -bq_r(head 2rt+1)
    bneg = nc.dram_tensor("bneg", [128, 4], F32, kind="ExternalInput").ap()
    # col rt: (bq_i - bq_r) in the same lane layout
    bdel = nc.dram_tensor("bdel", [128, 4], F32, kind="ExternalInput").ap()
    ytr = nc.dram_tensor("ytr", [E, S], F16, kind="ExternalOutput").ap()
    yti = nc.dram_tensor("yti", [E, S], F16, kind="ExternalOutput").ap()

    store = tc.alloc_tile_pool(name="store", bufs=1)
    qcat = store.tile([128, HPC * S], F16)    # per head j: [qr; qi]
    kcat1 = store.tile([128, HPC * S], F16)   # per head j: [kr; -ki]
    kcat2 = store.tile([128, HPC * S], F16)   # per head j: [ki; kr]
    vext = store.tile([128, 8 * HPC * VW], F16)
    bneg_sb = store.tile([128, 4], F32)
    bdel_sb = store.tile([128, 4], F32)
    nc.sync.dma_start(out=bneg_sb, in_=bneg)
    nc.sync.dma_start(out=bdel_sb, in_=bdel)

    def vr_view(st):  # [128, 8 heads, VW] view of vext for st-tile
        return vext.rearrange("p (t j w) -> p t j w", t=8, j=HPC, w=VW)[:, st]

    # ---------------- Q/K/V projections (one pool scope) --------------------
    with tc.tile_pool(name="xp", bufs=40) as xp, \
         tc.tile_pool(name="wp", bufs=16) as wp, \
         tc.tile_pool(name="sc", bufs=2) as sc, \
         tc.tile_pool(name="pp", bufs=8, space="PSUM") as pp:

        def phase_dmas(which, xdram, wdram, fine):
            """Emit this phase's DMAs + xs adds in consumption order.

            DMA-instruction issue costs ~600ns serially per HWDGE engine,
            so instructions alternate between the SP and Activation HWDGEs
            (2x issue rate) and stay coarse.  `fine` (Q phase) orders
            w-section0 / h0 halves first for a fast pipeline lead-in;
            other phases are prefetched far ahead and use whole tiles.
            """
            eng = (nc.sync, nc.scalar)
            wts = [wp.tile([128, 3 * EH], F16, tag="w", name=f"w{which}{k}")
                   for k in range(8)]
            xr = [xp.tile([128, S], F16, tag="x", name=f"{which}xr{k}")
                  for k in range(8)]
            xi = [xp.tile([128, S], F16, tag="x", name=f"{which}xi{k}")
                  for k in range(8)]
            xs = [xp.tile([128, S], F16, tag="x", name=f"{which}xs{k}")
                  for k in range(8)]
            if fine:
                for k in range(8):
                    r0 = k * 128
                    eng[k % 2].dma_start(out=wts[k][:, 0:512],
                                         in_=wdram[r0:r0 + 128, 0:512])
                # interleave xr/xi per k so the xs DVE adds (m3 feed) can
                # fire as early as possible during the warmup ramp
                for k in range(8):
                    r0 = k * 128
                    eng[k % 2].dma_start(out=xr[k][:, 0:512],
                                         in_=xdram[r0:r0 + 128, 0:512])
                    eng[(k + 1) % 2].dma_start(
                        out=xi[k][:, 0:512],
                        in_=xdram[E + r0:E + r0 + 128, 0:512])
                for k in range(8):
                    nc.vector.tensor_add(xs[k][:, 0:512], xr[k][:, 0:512],
                                         xi[k][:, 0:512])
                for s in (1, 2):
                    for k in range(8):
                        r0 = k * 128
                        eng[k % 2].dma_start(
                            out=wts[k][:, s * 512:(s + 1) * 512],
                            in_=wdram[r0:r0 + 128, s * 512:(s + 1) * 512])
                for k in range(8):
                    r0 = k * 128
                    eng[k % 2].dma_start(out=xr[k][:, 512:1024],
                                         in_=xdram[r0:r0 + 128, 512:1024])
                for k in range(8):
                    r0 = E + k * 128
                    eng[k % 2].dma_start(out=xi[k][:, 512:1024],
                                         in_=xdram[r0:r0 + 128, 512:1024])
                for k in range(8):
                    nc.vector.tensor_add(xs[k][:, 512:1024],
                                         xr[k][:, 512:1024],
                                         xi[k][:, 512:1024])
            else:
                # sync-only: a scalar-issued DMA emitted after this phase's
                # evacuation activations would park in the scalar queue and
                # stall them (in-order engine queues).
                for k in range(8):
                    r0 = k * 128
                    nc.sync.dma_start(out=wts[k],
                                      in_=wdram[r0:r0 + 128, :])
                for k in range(8):
                    r0 = k * 128
                    nc.sync.dma_start(out=xr[k],
                                      in_=xdram[r0:r0 + 128, :])
                for k in range(8):
                    r0 = E + k * 128
                    nc.sync.dma_start(out=xi[k],
                                      in_=xdram[r0:r0 + 128, :])
                for k in range(8):
                    nc.vector.tensor_add(xs[k], xr[k], xi[k])
            return wts, xr, xi, xs

        def qk_matmuls(which, wts, xr, xi, xs):
            # hf (sequence half) OUTER so h1 x data has a full half-phase
            # of matmul time to stream in behind the PE.
            for hf in range(2):
                rx = slice(hf * 512, (hf + 1) * 512)
                for rt in range(4):
                    je, jo = 2 * rt, 2 * rt + 1
                    c = rt * 128
                    m1 = pp.tile([128, 512], F32, tag="pp", bufs=8,
                                 name=f"{which}m1_{rt}{hf}")
                    m2 = pp.tile([128, 512], F32, tag="pp", bufs=8,
                                 name=f"{which}m2_{rt}{hf}")
                    m3 = pp.tile([128, 512], F32, tag="pp", bufs=8,
                                 name=f"{which}m3_{rt}{hf}")
                    for k in range(8):
                        nc.tensor.matmul(m1, wts[k][:, c:c + 128],
                                         xr[k][:, rx],
                                         start=(k == 0), stop=(k == 7))
                    for k in range(8):
                        nc.tensor.matmul(m2, wts[k][:, 512 + c:512 + c + 128],
                                         xi[k][:, rx],
                                         start=(k == 0), stop=(k == 7))
                    for k in range(8):
                        nc.tensor.matmul(m3, wts[k][:, 1024 + c:1024 + c + 128],
                                         xs[k][:, rx],
                                         start=(k == 0), stop=(k == 7))
                    m2s = sc.tile([128, 512], F32, tag="m2s", bufs=2,
                                  name=f"{which}m2s{rt}{hf}")
                    tms = sc.tile([128, 512], F32, tag="tms", bufs=2,
                                  name=f"{which}tms{rt}{hf}")
                    if which == "q":
                        nc.scalar.activation(m2s, m2, AF.Identity,
                                             bias=bneg_sb[:, rt:rt + 1])
                        nc.scalar.activation(tms, m3, AF.Identity,
                                             bias=bdel_sb[:, rt:rt + 1])
                    else:
                        nc.scalar.activation(m2s, m2, AF.Copy)
                        nc.scalar.activation(tms, m3, AF.Copy)
                    sv = sc.tile([128, 512], F32, tag="sv", bufs=2,
                                 name=f"{which}sv{rt}{hf}")
                    nc.vector.tensor_sub(sv, tms, m2s)   # M3 - M2 (+bias)
                    c0e = je * S + hf * 512
                    c0o = jo * S + hf * 512
                    if which == "q":
                        # qcat[j] = [qr; qi]
                        nc.vector.tensor_sub(qcat[0:64, c0e:c0e + 512],
                                             m1[0:64, :], m2s[0:64, :])
                        nc.vector.tensor_sub(qcat[0:64, c0o:c0o + 512],
                                             m1[64:128, :], m2s[64:128, :])
                        nc.vector.tensor_sub(qcat[64:128, c0e:c0e + 512],
                                             sv[0:64, :], m1[0:64, :])
                        nc.vector.tensor_sub(qcat[64:128, c0o:c0o + 512],
                                             sv[64:128, :], m1[64:128, :])
                    else:
                        # kcat1[j] = [kr; -ki], kcat2[j] = [ki; kr]
                        nc.vector.tensor_sub(kcat1[0:64, c0e:c0e + 512],
                                             m1[0:64, :], m2s[0:64, :])
                        nc.vector.tensor_sub(kcat1[0:64, c0o:c0o + 512],
                                             m1[64:128, :], m2s[64:128, :])
                        nc.vector.tensor_sub(kcat1[64:128, c0e:c0e + 512],
                                             m1[0:64, :], sv[0:64, :])
                        nc.vector.tensor_sub(kcat1[64:128, c0o:c0o + 512],
                                             m1[64:128, :], sv[64:128, :])
                        nc.vector.tensor_sub(kcat2[0:64, c0e:c0e + 512],
                                             sv[0:64, :], m1[0:64, :])
                        nc.vector.tensor_sub(kcat2[0:64, c0o:c0o + 512],
                                             sv[64:128, :], m1[64:128, :])
                        nc.vector.tensor_copy(kcat2[64:128, c0e:c0e + 512],
                                              kcat1[0:64, c0e:c0e + 512])
                        nc.vector.tensor_copy(kcat2[64:128, c0o:c0o + 512],
                                              kcat1[0:64, c0o:c0o + 512])

        # Q phase (fine lead-in pieces), then K, then V; each phase's DMAs
        # are emitted before its matmuls, so the SP DMA queue streams the
        # next phase's data while the PE chews on the current one.
        wtq, xrq, xiq, xsq = phase_dmas("q", xq, wq, fine=True)
        qk_matmuls("q", wtq, xrq, xiq, xsq)
        wtk, xrk, xik, xsk = phase_dmas("k", xk, wk, fine=False)
        qk_matmuls("k", wtk, xrk, xik, xsk)
        wtv, xrv, xiv, xsv = phase_dmas("v", xv, wv, fine=False)

        # ------------- V projection (Karatsuba, natural layout) -------------
        for st in range(8):
            m1 = pp.tile([128, 512], F32, tag="pp", bufs=8, name=f"vm1_{st}")
            m2 = pp.tile([128, 512], F32, tag="pp", bufs=8, name=f"vm2_{st}")
            m3 = pp.tile([128, 512], F32, tag="pp", bufs=8, name=f"vm3_{st}")
            cs = slice(st * 128, (st + 1) * 128)
            for k in range(8):
                nc.tensor.matmul(m1, xrv[k][:, cs], wtv[k][:, 0:512],
                                 start=(k == 0), stop=(k == 7))
            for k in range(8):
                nc.tensor.matmul(m2, xiv[k][:, cs], wtv[k][:, 512:1024],
                                 start=(k == 0), stop=(k == 7))
            for k in range(8):
                nc.tensor.matmul(m3, xsv[k][:, cs], wtv[k][:, 1024:1536],
                                 start=(k == 0), stop=(k == 7))
            # Vr = m1 - m2, Vi = m3 - m1 - m2
            m2s = sc.tile([128, 512], F32, tag="m2s", bufs=2, name=f"m2s{st}")
            nc.scalar.activation(m2s, m2, AF.Copy)
            sv = sc.tile([128, 512], F32, tag="sv", bufs=2, name=f"sv{st}")
            nc.vector.tensor_sub(sv, m3, m2s)          # m3 - m2
            vr = vr_view(st)
            m1v = m1.rearrange("p (j d) -> p j d", j=HPC)
            svv = sv.rearrange("p (j d) -> p j d", j=HPC)
            m2v = m2s.rearrange("p (j d) -> p j d", j=HPC)
            nc.vector.tensor_sub(vr[:, :, 1:65], m1v, m2v)
            nc.vector.tensor_sub(vr[:, :, 65:129], svv, m1v)
            nc.vector.memset(vr[:, :, 0:1], 1.0)

    # ---------------- O-projection weight prefetch --------------------------
    # (fresh pool; DMAs drain during late projections / early attention)
    wop = tc.alloc_tile_pool(name="wop", bufs=1)
    wo_sb = wop.tile([128, 16, 8, 128], F16)
    for i in range(16):
        nc.sync.dma_start(out=wo_sb[:, i],
                          in_=wot[:, i * 1024:(i + 1) * 1024])
    afp = tc.alloc_tile_pool(name="afp", bufs=1)
    attn_fs = afp.tile([128, HPC * S], F16)  # per head j: [or_d; oi_d] x q

    # ---------------- attention (qi outer) + inline O projection ------------
    with tc.tile_pool(name="asb", bufs=2) as asb, \
         tc.tile_pool(name="stp", bufs=2, space="PSUM") as stp, \
         tc.tile_pool(name="avp", bufs=4, space="PSUM") as avp, \
         tc.tile_pool(name="ytp", bufs=4) as ytp:

        def emit_oproj_piece(qi, part, m, last=False):
            sq0 = qi * 512
            yt_d = ytr if part == 0 else yti
            wt = wo_sb[:, part * 8 + m]
            pso = avp.tile([128, 512], F32, tag="av", name=f"po{part}{m}{qi}")
            for jj in range(HPC):
                nc.tensor.matmul(
                    pso, wt[:, jj, :],
                    attn_fs[:, jj * S + sq0: jj * S + sq0 + 512],
                    start=(jj == 0), stop=(jj == 7))
            yt_t = ytp.tile([128, 512], F16, tag="yt", name=f"yt{part}{m}{qi}")
            nc.vector.tensor_copy(yt_t, pso)
            if last:
                # split across both HWDGEs for tail latency (the scalar
                # exp stream is already done at this point)
                nc.sync.dma_start(
                    out=yt_d[m * 128:(m + 1) * 128, sq0:sq0 + 256],
                    in_=yt_t[:, 0:256])
                nc.scalar.dma_start(
                    out=yt_d[m * 128:(m + 1) * 128, sq0 + 256:sq0 + 512],
                    in_=yt_t[:, 256:512])
            else:
                nc.sync.dma_start(
                    out=yt_d[m * 128:(m + 1) * 128, sq0:sq0 + 512],
                    in_=yt_t)

        def oslice(bks, ri, qc):
            bankR, bankI, bankM = bks
            if qc < 3:
                b = bankR if ri == 0 else bankI
                return b[:, qc * 129:qc * 129 + 129]
            return bankM[:, ri * 129:ri * 129 + 129]

        def emit_scores(qi, j, t):
            sq0 = qi * 512
            qh = qcat[:, j * S + sq0: j * S + sq0 + 512]
            # one 2-bank tile: r scores in [:,0:512], i in [:,512:]
            st_t = stp.tile([128, 1024], F32, tag="st", name=f"st{j}{qi}{t}")
            kc = j * S + t * 128
            nc.tensor.matmul(st_t[:, 0:512], kcat1[:, kc:kc + 128],
                             qh, start=True, stop=True)
            nc.tensor.matmul(st_t[:, 512:1024], kcat2[:, kc:kc + 128],
                             qh, start=True, stop=True)
            pt = asb.tile([128, 1024], F16, tag="pt", bufs=3,
                          name=f"pt{j}{qi}{t}")
            nc.scalar.activation(pt, st_t, AF.Exp, scale=0.125)
            return pt

        def emit_av(qi, j, t, pt, bks):
            vrt = vr_view(t)[:, j, 0:129]
            # start=True zeroes the whole 2KB bank: only the first
            # group per bank starts; only the last group stops.
            for qc in range(4):
                nc.tensor.matmul(oslice(bks, 0, qc),
                                 pt[:, qc * 128:(qc + 1) * 128], vrt,
                                 start=(t == 0 and qc in (0, 3)),
                                 stop=(t == 7 and qc == 2))
                nc.tensor.matmul(oslice(bks, 1, qc),
                                 pt[:, 512 + qc * 128: 512 + (qc + 1) * 128],
                                 vrt,
                                 start=(t == 0 and qc == 0),
                                 stop=(t == 7 and qc in (2, 3)))

        def emit_finalize(qi, j, bks):
            # finalize: rcp (DVE), normalize on DVE r-part first (frees the
            # PSUM banks for the next head's AV in WAR-chain order),
            # combine (gpsimd), transpose (DMA xbar on the Sync engine).
            sq0 = qi * 512
            bankR, bankI, bankM = bks
            rcp = asb.tile([128, 2, 4], F32, tag="rcp", name=f"rcp{j}{qi}")
            bRv = bankR[:, 0:387].rearrange("p (g w) -> p g w", w=129, g=3)
            bIv = bankI[:, 0:387].rearrange("p (g w) -> p g w", w=129, g=3)
            bMv = bankM[:, 0:258].rearrange("p (g w) -> p g w", w=129, g=2)
            nc.vector.reciprocal(rcp[:, 0, 0:3], bRv[:, :, 0])
            nc.vector.reciprocal(rcp[:, 1, 0:3], bIv[:, :, 0])
            nc.vector.reciprocal(rcp[:, :, 3], bMv[:, :, 0])
            tmp_r = asb.tile([128, 4, 128], F16, tag="tmr", name=f"tr{j}{qi}")
            tmp_i = asb.tile([128, 4, 128], F16, tag="tmi", name=f"ti{j}{qi}")
            for qc in range(4):
                nc.vector.tensor_scalar_mul(
                    tmp_r[:, qc, :], oslice(bks, 0, qc)[:, 1:129],
                    rcp[:, 0, qc:qc + 1])
            for qc in range(4):
                nc.vector.tensor_scalar_mul(
                    tmp_i[:, qc, :], oslice(bks, 1, qc)[:, 1:129],
                    rcp[:, 1, qc:qc + 1])
            # tmp_r = [PrVr | PrVi], tmp_i = [PiVr | PiVi] (both from the
            # same [1|Vr|Vi] tile): real = PrVr - PiVi crosses halves.
            attn_sb = asb.tile([128, 4, 128], F16, tag="ats", name=f"as{j}{qi}")
            nc.gpsimd.tensor_sub(attn_sb[:, :, 0:64], tmp_r[:, :, 0:64],
                                 tmp_i[:, :, 64:128])
            nc.gpsimd.tensor_add(attn_sb[:, :, 64:128], tmp_r[:, :, 64:128],
                                 tmp_i[:, :, 0:64])
            # DMA transposes run ~1.2us each on the issuing ENGINE, so keep
            # them whole; the last head's four (tail critical path) split
            # across the two HWDGE engines (exp stream is done by then).
            last = (qi == 1 and j == HPC - 1)
            for qc in range(4):
                c0 = j * S + sq0 + qc * 128
                e = nc.scalar if (last and qc % 2) else nc.sync
                e.dma_start_transpose(
                    attn_fs[:, c0:c0 + 128], attn_sb[:, qc, :])

        # Per-head software pipeline: scores/exp one t-stage ahead of AV so
        # the in-order engine queues keep the PE busy during exp.
        for qi in range(2):
            for j in range(HPC):
                bks = (avp.tile([128, 512], F32, tag="av", name=f"bR{j}{qi}"),
                       avp.tile([128, 512], F32, tag="av", name=f"bI{j}{qi}"),
                       avp.tile([128, 512], F32, tag="av", name=f"bM{j}{qi}"))
                pts = {}
                for t in range(9):
                    if t < 8:
                        pts[t] = emit_scores(qi, j, t)
                    if t == 0:
                        continue
                    emit_av(qi, j, t - 1, pts.pop(t - 1), bks)
                emit_finalize(qi, j, bks)
                if qi == 1:
                    # interleave qi=0's O projection with qi=1's attention so
                    # O-proj matmuls fill the PE while exp runs on scalar.
                    for p in (2 * j, 2 * j + 1):
                        emit_oproj_piece(0, p // 8, p % 8)

        # tail: O projection for qi=1
        for part in range(2):
            for m in range(8):
                emit_oproj_piece(1, part, m, last=(part == 1 and m >= 6))

    afp.release()
    wop.release()
    store.release()


def build_module():
    nc = bacc.Bacc("TRN2", target_bir_lowering=False)
    with tile.TileContext(nc) as tc:
        _emit(tc)
    nc.compile()
    return nc


def _get_nc():
    if not _NC_CACHE:
        _NC_CACHE.append(build_module())
    return _NC_CACHE[0]


def prep_core(inp, core):
    """Host-side shard prep for one core."""
    b, hg = divmod(core, 2)
    hs, he = hg * EH, (hg + 1) * EH

    def xcat2(xr, xi):
        a = xr[:, b, :].T.astype(np.float16)
        c = xi[:, b, :].T.astype(np.float16)
        return np.ascontiguousarray(np.concatenate([a, c], axis=0))

    def wv_prep(wr, wi):
        A = wr[hs:he, :].T.astype(np.float32)
        Bm = wi[hs:he, :].T.astype(np.float32)
        return np.ascontiguousarray(
            np.concatenate([A, Bm, A + Bm], axis=1), dtype=np.float16)

    def wo_prep(w_top, w_bot):
        Ct = w_top[:, hs:he].T.reshape(HPC, D, E)
        Dt = w_bot[:, hs:he].T.reshape(HPC, D, E)
        return np.concatenate([Ct, Dt], axis=1).reshape(2 * EH, E)

    # wot[p, part, m, j, n] = wo_part[j*128 + p, m*128 + n]
    wor = wo_prep(inp["wo_r"], -inp["wo_i"])
    woi = wo_prep(inp["wo_i"], inp["wo_r"])
    wo_st = np.stack([wor, woi])                      # [2, 1024, 1024]
    wo_st = wo_st.reshape(2, 8, 128, 8, 128)          # [part, j, p, m, n]
    wot = np.ascontiguousarray(
        wo_st.transpose(2, 0, 3, 1, 4).reshape(128, 16 * 8 * 128),
        dtype=np.float16)

    bneg = np.empty((128, 4), np.float32)
    bdel = np.empty((128, 4), np.float32)
    for rt in range(4):
        for par in range(2):
            h = hg * HPC + 2 * rt + par
            sl = slice(par * 64, (par + 1) * 64)
            br = inp["bq_r"][h * D:(h + 1) * D]
            bi = inp["bq_i"][h * D:(h + 1) * D]
            bneg[sl, rt] = -br
            bdel[sl, rt] = bi - br

    return dict(
        xq=xcat2(inp["query_r"], inp["query_i"]),
        xk=xcat2(inp["key_r"], inp["key_i"]),
        xv=xcat2(inp["value_r"], inp["value_i"]),
        wq=wv_prep(inp["wq_r"], inp["wq_i"]),
        wk=wv_prep(inp["wk_r"], inp["wk_i"]),
        wv=wv_prep(inp["wv_r"], inp["wv_i"]),
        wot=wot,
        bneg=bneg,
        bdel=bdel,
    )


def host_combine(results, inp):
    """Sum per-core partials, add the host-side constant, untranspose."""
    bvr = inp["bv_r"].astype(np.float64)
    bvi = inp["bv_i"].astype(np.float64)
    wr = inp["wo_r"].astype(np.float64)
    wi = inp["wo_i"].astype(np.float64)
    vb_r = bvr - bvi
    vb_i = bvr + bvi
    yc_r = (wr @ vb_r - wi @ vb_i + inp["bo_r"]).astype(np.float32)
    yc_i = (wr @ vb_i + wi @ vb_r + inp["bo_i"]).astype(np.float32)

    out = np.empty((S, B, E, 2), np.float32)
    for b in range(B):
        yr = (results[2 * b]["ytr"].astype(np.float32)
              + results[2 * b + 1]["ytr"].astype(np.float32))
        yi = (results[2 * b]["yti"].astype(np.float32)
              + results[2 * b + 1]["yti"].astype(np.float32))
        out[:, b, :, 0] = yr.T + yc_r
        out[:, b, :, 1] = yi.T + yc_i
    return out


def kernel(**inputs):
    inputs = {k: np.asarray(v) for k, v in inputs.items()}
    nc = _get_nc()
    in_maps = [prep_core(inputs, c) for c in range(N_CORES)]
    res = run_bass_kernel_spmd(nc, in_maps, core_ids=list(range(N_CORES)))
    return host_combine(res.results, inputs)


# revision 38
# speedup vs baseline: 1.1667x; 1.1667x over previous
"""Complex multihead attention v3: fp16 PE, V-Karatsuba, transposed AV with
fused softmax sums, DMA transposes, qi-outer loop with inline O projection.

Sharding: data-parallel over batch (B=4) x tensor-parallel over heads
(16 heads -> 2 groups of 8). core = b*2 + head_group. Host combines partials.

Math notes (validated against reference):
 - K bias dropped (softmax invariant); V bias folded to host constant;
   Q bias applied at PSUM evacuation (per-partition ACT bias).
 - Q projection: stacked-real trick -> qcat = [qr; qi] per head.
 - K projection: same stacked psum [kr; ki], evacuated TWICE with
   cross-partition-offset activations: kcat1 = [kr; -ki], kcat2 = [ki; kr],
   so scores are st_r = kcat1^T qcat, st_i = kcat2^T qcat (no per-head
   query prep in the attention phase).
 - Q/K/V projections: Karatsuba M1=xr*A, M2=xi*B, M3=(xr+xi)*(A+B) with
   xs = xr+xi computed on-device (DVE) to cut x DMA traffic by 1/3.
 - Scores transposed [k, q]; exp on scalar engine (fp16 out).
 - AV transposed: out[q, f] = P[:,qc]^T @ [1|Vr|Vi]: softmax sums ride in
   col 0; normalizer = per-partition activation scale. Both r and i P parts
   multiply the SAME [1|Vr|Vi] tile (no [1|Vi|Vr] mirror): bankR = [s|PrVr|
   PrVi], bankI = [s|PiVr|PiVi]; combine crosses halves. attn [q,f] -> [f,q]
   via DMA xbar transpose (split in partition halves for latency).
 - O projection per qi-half inline (overlaps the other half's attention).
 - DMAs: consumption-ordered, section/half granularity; 40-buf x pool so
   next-phase x prefetch streams behind the current phase's matmuls.
"""

import numpy as np

import concourse.bass as bass
from concourse import bacc
import concourse.mybir as mybir
import concourse.tile as tile
from concourse.bass_utils import run_bass_kernel_spmd

S, B, E, H, D = 1024, 4, 1024, 16, 64
HPC = 8            # heads per core
EH = HPC * D       # 512
N_CORES = 8
F32 = mybir.dt.float32
F16 = mybir.dt.float16
AF = mybir.ActivationFunctionType

_NC_CACHE = []

VW = 130           # vext per-head stride ([1|Vr|Vi] = 129, +1 pad)


def _emit(tc):
    nc = tc.nc
    # x payload: rows 0:1024 = x_r^T, rows 1024:2048 = x_i^T (xs on-device)
    xq = nc.dram_tensor("xq", [2 * E, S], F16, kind="ExternalInput").ap()
    xk = nc.dram_tensor("xk", [2 * E, S], F16, kind="ExternalInput").ap()
    xv = nc.dram_tensor("xv", [2 * E, S], F16, kind="ExternalInput").ap()
    wq = nc.dram_tensor("wq", [E, 3 * EH], F16, kind="ExternalInput").ap()
    wk = nc.dram_tensor("wk", [E, 3 * EH], F16, kind="ExternalInput").ap()
    wv = nc.dram_tensor("wv", [E, 3 * EH], F16, kind="ExternalInput").ap()
    # O-proj weights pre-swizzled on host: [128, (part,m,j,n)] so each
    # (part,m) chunk is one contiguous 2KB-per-partition DMA.
    wot = nc.dram_tensor("wot", [128, 16 * 8 * 128], F16,
                         kind="ExternalInput").ap()
    # col rt: lanes 0:64 = -bq_r(head 2rt), 64:128 = -bq_r(head 2rt+1)
    bneg = nc.dram_tensor("bneg", [128, 4], F32, kind="ExternalInput").ap()
    # col rt: (bq_i - bq_r) in the same lane layout
    bdel = nc.dram_tensor("bdel", [128, 4], F32, kind="ExternalInput").ap()
    ytr = nc.dram_tensor("ytr", [E, S], F16, kind="ExternalOutput").ap()
    yti = nc.dram_tensor("yti", [E, S], F16, kind="ExternalOutput").ap()

    store = tc.alloc_tile_pool(name="store", bufs=1)
    qcat = store.tile([128, HPC * S], F16)    # per head j: [qr; qi]
    kcat1 = store.tile([128, HPC * S], F16)   # per head j: [kr; -ki]
    kcat2 = store.tile([128, HPC * S], F16)   # per head j: [ki; kr]
    vext = store.tile([128, 8 * HPC * VW], F16)
    bneg_sb = store.tile([128, 4], F32)
    bdel_sb = store.tile([128, 4], F32)
    nc.sync.dma_start(out=bneg_sb, in_=bneg)
    nc.sync.dma_start(out=bdel_sb, in_=bdel)

    def vr_view(st):  # [128, 8 heads, VW] view of vext for st-tile
        return vext.rearrange("p (t j w) -> p t j w", t=8, j=HPC, w=VW)[:, st]

    # ---------------- Q/K/V projections (one pool scope) --------------------
    with tc.tile_pool(name="xp", bufs=40) as xp, \
         tc.tile_pool(name="wp", bufs=16) as wp, \
         tc.tile_pool(name="sc", bufs=2) as sc, \
         tc.tile_pool(name="pp", bufs=8, space="PSUM") as pp:

        def phase_dmas(which, xdram, wdram, fine):
            """Emit this phase's DMAs + xs adds in consumption order.

            DMA-instruction issue costs ~600ns serially per HWDGE engine,
            so instructions alternate between the SP and Activation HWDGEs
            (2x issue rate) and stay coarse.  `fine` (Q phase) orders
            w-section0 / h0 halves first for a fast pipeline lead-in;
            other phases are prefetched far ahead and use whole tiles.
            """
            eng = (nc.sync, nc.scalar)
            wts = [wp.tile([128, 3 * EH], F16, tag="w", name=f"w{which}{k}")
                   for k in range(8)]
            xr = [xp.tile([128, S], F16, tag="x", name=f"{which}xr{k}")
                  for k in range(8)]
            xi = [xp.tile([128, S], F16, tag="x", name=f"{which}xi{k}")
                  for k in range(8)]
            xs = [xp.tile([128, S], F16, tag="x", name=f"{which}xs{k}")
                  for k in range(8)]
            if fine:
                for k in range(8):
                    r0 = k * 128
                    eng[k % 2].dma_start(out=wts[k][:, 0:512],
                                         in_=wdram[r0:r0 + 128, 0:512])
                # interleave xr/xi per k so the xs DVE adds (m3 feed) can
                # fire as early as possible during the warmup ramp
                for k in range(8):
                    r0 = k * 128
                    eng[k % 2].dma_start(out=xr[k][:, 0:512],
                                         in_=xdram[r0:r0 + 128, 0:512])
                    eng[(k + 1) % 2].dma_start(
                        out=xi[k][:, 0:512],
                        in_=xdram[E + r0:E + r0 + 128, 0:512])
                for k in range(8):
                    nc.vector.tensor_add(xs[k][:, 0:512], xr[k][:, 0:512],
                                         xi[k][:, 0:512])
                for s in (1, 2):
                    for k in range(8):
                        r0 = k * 128
                        eng[k % 2].dma_start(
                            out=wts[k][:, s * 512:(s + 1) * 512],
                            in_=wdram[r0:r0 + 128, s * 512:(s + 1) * 512])
                for k in range(8):
                    r0 = k * 128
                    eng[k % 2].dma_start(out=xr[k][:, 512:1024],
                                         in_=xdram[r0:r0 + 128, 512:1024])
                for k in range(8):
                    r0 = E + k * 128
                    eng[k % 2].dma_start(out=xi[k][:, 512:1024],
                                         in_=xdram[r0:r0 + 128, 512:1024])
                for k in range(8):
                    nc.vector.tensor_add(xs[k][:, 512:1024],
                                         xr[k][:, 512:1024],
                                         xi[k][:, 512:1024])
            else:
                # sync-only: a scalar-issued DMA emitted after this phase's
                # evacuation activations would park in the scalar queue and
                # stall them (in-order engine queues).
                for k in range(8):
                    r0 = k * 128
                    nc.sync.dma_start(out=wts[k],
                                      in_=wdram[r0:r0 + 128, :])
                for k in range(8):
                    r0 = k * 128
                    nc.sync.dma_start(out=xr[k],
                                      in_=xdram[r0:r0 + 128, :])
                for k in range(8):
                    r0 = E + k * 128
                    nc.sync.dma_start(out=xi[k],
                                      in_=xdram[r0:r0 + 128, :])
                for k in range(8):
                    nc.vector.tensor_add(xs[k], xr[k], xi[k])
            return wts, xr, xi, xs

        def qk_matmuls(which, wts, xr, xi, xs):
            # hf (sequence half) OUTER so h1 x data has a full half-phase
            # of matmul time to stream in behind the PE.
            for hf in range(2):
                rx = slice(hf * 512, (hf + 1) * 512)
                for rt in range(4):
                    je, jo = 2 * rt, 2 * rt + 1
                    c = rt * 128
                    m1 = pp.tile([128, 512], F32, tag="pp", bufs=8,
                                 name=f"{which}m1_{rt}{hf}")
                    m2 = pp.tile([128, 512], F32, tag="pp", bufs=8,
                                 name=f"{which}m2_{rt}{hf}")
                    m3 = pp.tile([128, 512], F32, tag="pp", bufs=8,
                                 name=f"{which}m3_{rt}{hf}")
                    for k in range(8):
                        nc.tensor.matmul(m1, wts[k][:, c:c + 128],
                                         xr[k][:, rx],
                                         start=(k == 0), stop=(k == 7))
                    for k in range(8):
                        nc.tensor.matmul(m2, wts[k][:, 512 + c:512 + c + 128],
                                         xi[k][:, rx],
                                         start=(k == 0), stop=(k == 7))
                    for k in range(8):
                        nc.tensor.matmul(m3, wts[k][:, 1024 + c:1024 + c + 128],
                                         xs[k][:, rx],
                                         start=(k == 0), stop=(k == 7))
                    m2s = sc.tile([128, 512], F32, tag="m2s", bufs=2,
                                  name=f"{which}m2s{rt}{hf}")
                    tms = sc.tile([128, 512], F32, tag="tms", bufs=2,
                                  name=f"{which}tms{rt}{hf}")
                    if which == "q":
                        nc.scalar.activation(m2s, m2, AF.Identity,
                                             bias=bneg_sb[:, rt:rt + 1])
                        nc.scalar.activation(tms, m3, AF.Identity,
                                             bias=bdel_sb[:, rt:rt + 1])
                    else:
                        nc.scalar.activation(m2s, m2, AF.Copy)
                        nc.scalar.activation(tms, m3, AF.Copy)
                    sv = sc.tile([128, 512], F32, tag="sv", bufs=2,
                                 name=f"{which}sv{rt}{hf}")
                    nc.vector.tensor_sub(sv, tms, m2s)   # M3 - M2 (+bias)
                    c0e = je * S + hf * 512
                    c0o = jo * S + hf * 512
                    if which == "q":
                        # qcat[j] = [qr; qi]
                        nc.vector.tensor_sub(qcat[0:64, c0e:c0e + 512],
                                             m1[0:64, :], m2s[0:64, :])
                        nc.vector.tensor_sub(qcat[0:64, c0o:c0o + 512],
                                             m1[64:128, :], m2s[64:128, :])
                        nc.vector.tensor_sub(qcat[64:128, c0e:c0e + 512],
                                             sv[0:64, :], m1[0:64, :])
                        nc.vector.tensor_sub(qcat[64:128, c0o:c0o + 512],
                                             sv[64:128, :], m1[64:128, :])
                    else:
                        # kcat1[j] = [kr; -ki], kcat2[j] = [ki; kr]
                        nc.vector.tensor_sub(kcat1[0:64, c0e:c0e + 512],
                                             m1[0:64, :], m2s[0:64, :])
                        nc.vector.tensor_sub(kcat1[0:64, c0o:c0o + 512],
                                             m1[64:128, :], m2s[64:128, :])
                        nc.vector.tensor_sub(kcat1[64:128, c0e:c0e + 512],
                                             m1[0:64, :], sv[0:64, :])
                        nc.vector.tensor_sub(kcat1[64:128, c0o:c0o + 512],
                                             m1[64:128, :], sv[64:128, :])
                        nc.vector.tensor_sub(kcat2[0:64, c0e:c0e + 512],
                                             sv[0:64, :], m1[0:64, :])
                        nc.vector.tensor_sub(kcat2[0:64, c0o:c0o + 512],
                                             sv[64:128, :], m1[64:128, :])
                        nc.vector.tensor_copy(kcat2[64:128, c0e:c0e + 512],
                                              kcat1[0:64, c0e:c0e + 512])
                        nc.vector.tensor_copy(kcat2[64:128, c0o:c0o + 512],
                                              kcat1[0:64, c0o:c0o + 512])

        # Q phase (fine lead-in pieces), then K, then V; each phase's DMAs
        # are emitted before its matmuls, so the SP DMA queue streams the
        # next phase's data while the PE chews on the current one.
        wtq, xrq, xiq, xsq = phase_dmas("q", xq, wq, fine=True)
        qk_matmuls("q", wtq, xrq, xiq, xsq)
        wtk, xrk, xik, xsk = phase_dmas("k", xk, wk, fine=False)
        qk_matmuls("k", wtk, xrk, xik, xsk)
        wtv, xrv, xiv, xsv = phase_dmas("v", xv, wv, fine=False)

        # ------------- V projection (Karatsuba, natural layout) -------------
        for st in range(8):
            m1 = pp.tile([128, 512], F32, tag="pp", bufs=8, name=f"vm1_{st}")
            m2 = pp.tile([128, 512], F32, tag="pp", bufs=8, name=f"vm2_{st}")
            m3 = pp.tile([128, 512], F32, tag="pp", bufs=8, name=f"vm3_{st}")
            cs = slice(st * 128, (st + 1) * 128)
            for k in range(8):
                nc.tensor.matmul(m1, xrv[k][:, cs], wtv[k][:, 0:512],
                                 start=(k == 0), stop=(k == 7))
            for k in range(8):
                nc.tensor.matmul(m2, xiv[k][:, cs], wtv[k][:, 512:1024],
                                 start=(k == 0), stop=(k == 7))
            for k in range(8):
                nc.tensor.matmul(m3, xsv[k][:, cs], wtv[k][:, 1024:1536],
                                 start=(k == 0), stop=(k == 7))
            # Vr = m1 - m2, Vi = m3 - m1 - m2
            m2s = sc.tile([128, 512], F32, tag="m2s", bufs=2, name=f"m2s{st}")
            nc.scalar.activation(m2s, m2, AF.Copy)
            sv = sc.tile([128, 512], F32, tag="sv", bufs=2, name=f"sv{st}")
            nc.vector.tensor_sub(sv, m3, m2s)          # m3 - m2
            vr = vr_view(st)
            m1v = m1.rearrange("p (j d) -> p j d", j=HPC)
            svv = sv.rearrange("p (j d) -> p j d", j=HPC)
            m2v = m2s.rearrange("p (j d) -> p j d", j=HPC)
            nc.vector.tensor_sub(vr[:, :, 1:65], m1v, m2v)
            nc.vector.tensor_sub(vr[:, :, 65:129], svv, m1v)
            nc.vector.memset(vr[:, :, 0:1], 1.0)

    # ---------------- O-projection weight prefetch --------------------------
    # (fresh pool; DMAs drain during late projections / early attention)
    wop = tc.alloc_tile_pool(name="wop", bufs=1)
    wo_sb = wop.tile([128, 16, 8, 128], F16)
    for i in range(16):
        nc.sync.dma_start(out=wo_sb[:, i],
                          in_=wot[:, i * 1024:(i + 1) * 1024])
    afp = tc.alloc_tile_pool(name="afp", bufs=1)
    attn_fs = afp.tile([128, HPC * S], F16)  # per head j: [or_d; oi_d] x q

    # ---------------- attention (qi outer) + inline O projection ------------
    with tc.tile_pool(name="asb", bufs=2) as asb, \
         tc.tile_pool(name="stp", bufs=2, space="PSUM") as stp, \
         tc.tile_pool(name="avp", bufs=4, space="PSUM") as avp, \
         tc.tile_pool(name="ytp", bufs=4) as ytp:

        def emit_oproj_piece(qi, part, m, last=False):
            sq0 = qi * 512
            yt_d = ytr if part == 0 else yti
            wt = wo_sb[:, part * 8 + m]
            pso = avp.tile([128, 512], F32, tag="av", name=f"po{part}{m}{qi}")
            for jj in range(HPC):
                nc.tensor.matmul(
                    pso, wt[:, jj, :],
                    attn_fs[:, jj * S + sq0: jj * S + sq0 + 512],
                    start=(jj == 0), stop=(jj == 7))
            yt_t = ytp.tile([128, 512], F16, tag="yt", name=f"yt{part}{m}{qi}")
            # evacuate on the scalar engine: it has slack here (exp stream
            # is ahead/done), and this frees the shared "av"-tag PSUM bank
            # without queueing behind the finalize's DVE ops
            nc.scalar.activation(yt_t, pso, AF.Copy)
            if last:
                # split across both HWDGEs for tail latency (the scalar
                # exp stream is already done at this point)
                nc.sync.dma_start(
                    out=yt_d[m * 128:(m + 1) * 128, sq0:sq0 + 256],
                    in_=yt_t[:, 0:256])
                nc.scalar.dma_start(
                    out=yt_d[m * 128:(m + 1) * 128, sq0 + 256:sq0 + 512],
                    in_=yt_t[:, 256:512])
            else:
                nc.sync.dma_start(
                    out=yt_d[m * 128:(m + 1) * 128, sq0:sq0 + 512],
                    in_=yt_t)

        def oslice(bks, ri, qc):
            bankR, bankI, bankM = bks
            if qc < 3:
                b = bankR if ri == 0 else bankI
                return b[:, qc * 129:qc * 129 + 129]
            return bankM[:, ri * 129:ri * 129 + 129]

        def emit_scores(qi, j, t):
            sq0 = qi * 512
            qh = qcat[:, j * S + sq0: j * S + sq0 + 512]
            # one 2-bank tile: r scores in [:,0:512], i in [:,512:]
            st_t = stp.tile([128, 1024], F32, tag="st", name=f"st{j}{qi}{t}")
            kc = j * S + t * 128
            nc.tensor.matmul(st_t[:, 0:512], kcat1[:, kc:kc + 128],
                             qh, start=True, stop=True)
            nc.tensor.matmul(st_t[:, 512:1024], kcat2[:, kc:kc + 128],
                             qh, start=True, stop=True)
            pt = asb.tile([128, 1024], F16, tag="pt", bufs=3,
                          name=f"pt{j}{qi}{t}")
            nc.scalar.activation(pt, st_t, AF.Exp, scale=0.125)
            return pt

        def emit_av(qi, j, t, pt, bks):
            vrt = vr_view(t)[:, j, 0:129]
            # start=True zeroes the whole 2KB bank: only the first
            # group per bank starts; only the last group stops.
            for qc in range(4):
                nc.tensor.matmul(oslice(bks, 0, qc),
                                 pt[:, qc * 128:(qc + 1) * 128], vrt,
                                 start=(t == 0 and qc in (0, 3)),
                                 stop=(t == 7 and qc == 2))
                nc.tensor.matmul(oslice(bks, 1, qc),
                                 pt[:, 512 + qc * 128: 512 + (qc + 1) * 128],
                                 vrt,
                                 start=(t == 0 and qc == 0),
                                 stop=(t == 7 and qc in (2, 3)))

        def emit_finalize(qi, j, bks):
            # finalize: rcp (DVE), normalize on DVE r-part first (frees the
            # PSUM banks for the next head's AV in WAR-chain order),
            # combine (gpsimd), transpose (DMA xbar on the Sync engine).
            sq0 = qi * 512
            bankR, bankI, bankM = bks
            rcp = asb.tile([128, 2, 4], F32, tag="rcp", name=f"rcp{j}{qi}")
            bRv = bankR[:, 0:387].rearrange("p (g w) -> p g w", w=129, g=3)
            bIv = bankI[:, 0:387].rearrange("p (g w) -> p g w", w=129, g=3)
            bMv = bankM[:, 0:258].rearrange("p (g w) -> p g w", w=129, g=2)
            nc.vector.reciprocal(rcp[:, 0, 0:3], bRv[:, :, 0])
            nc.vector.reciprocal(rcp[:, 1, 0:3], bIv[:, :, 0])
            nc.vector.reciprocal(rcp[:, :, 3], bMv[:, :, 0])
            tmp_r = asb.tile([128, 4, 128], F16, tag="tmr", name=f"tr{j}{qi}")
            tmp_i = asb.tile([128, 4, 128], F16, tag="tmi", name=f"ti{j}{qi}")
            for qc in range(4):
                nc.vector.tensor_scalar_mul(
                    tmp_r[:, qc, :], oslice(bks, 0, qc)[:, 1:129],
                    rcp[:, 0, qc:qc + 1])
            for qc in range(4):
                nc.vector.tensor_scalar_mul(
                    tmp_i[:, qc, :], oslice(bks, 1, qc)[:, 1:129],
                    rcp[:, 1, qc:qc + 1])
            # tmp_r = [PrVr | PrVi], tmp_i = [PiVr | PiVi] (both from the
            # same [1|Vr|Vi] tile): real = PrVr - PiVi crosses halves.
            attn_sb = asb.tile([128, 4, 128], F16, tag="ats", name=f"as{j}{qi}")
            nc.gpsimd.tensor_sub(attn_sb[:, :, 0:64], tmp_r[:, :, 0:64],
                                 tmp_i[:, :, 64:128])
            nc.gpsimd.tensor_add(attn_sb[:, :, 64:128], tmp_r[:, :, 64:128],
                                 tmp_i[:, :, 0:64])
            # DMA transposes run ~1.2us each on the issuing ENGINE, so keep
            # them whole; the last head's four (tail critical path) split
            # across the two HWDGE engines (exp stream is done by then).
            last = (qi == 1 and j == HPC - 1)
            for qc in range(4):
                c0 = j * S + sq0 + qc * 128
                e = nc.scalar if (last and qc % 2) else nc.sync
                e.dma_start_transpose(
                    attn_fs[:, c0:c0 + 128], attn_sb[:, qc, :])

        # Per-head software pipeline: scores/exp one t-stage ahead of AV so
        # the in-order engine queues keep the PE busy during exp.
        for qi in range(2):
            for j in range(HPC):
                bks = (avp.tile([128, 512], F32, tag="av", name=f"bR{j}{qi}"),
                       avp.tile([128, 512], F32, tag="av", name=f"bI{j}{qi}"),
                       avp.tile([128, 512], F32, tag="av", name=f"bM{j}{qi}"))
                pts = {}
                for t in range(9):
                    if t < 8:
                        pts[t] = emit_scores(qi, j, t)
                    if t == 0:
                        continue
                    emit_av(qi, j, t - 1, pts.pop(t - 1), bks)
                emit_finalize(qi, j, bks)
                if qi == 1:
                    # interleave qi=0's O projection with qi=1's attention so
                    # O-proj matmuls fill the PE while exp runs on scalar.
                    for p in (2 * j, 2 * j + 1):
                        emit_oproj_piece(0, p // 8, p % 8)

        # tail: O projection for qi=1
        for part in range(2):
            for m in range(8):
                emit_oproj_piece(1, part, m, last=(part == 1 and m >= 6))

    afp.release()
    wop.release()
    store.release()


def build_module():
    nc = bacc.Bacc("TRN2", target_bir_lowering=False)
    with tile.TileContext(nc) as tc:
        _emit(tc)
    nc.compile()
    return nc


def _get_nc():
    if not _NC_CACHE:
        _NC_CACHE.append(build_module())
    return _NC_CACHE[0]


def prep_core(inp, core):
    """Host-side shard prep for one core."""
    b, hg = divmod(core, 2)
    hs, he = hg * EH, (hg + 1) * EH

    def xcat2(xr, xi):
        a = xr[:, b, :].T.astype(np.float16)
        c = xi[:, b, :].T.astype(np.float16)
        return np.ascontiguousarray(np.concatenate([a, c], axis=0))

    def wv_prep(wr, wi):
        A = wr[hs:he, :].T.astype(np.float32)
        Bm = wi[hs:he, :].T.astype(np.float32)
        return np.ascontiguousarray(
            np.concatenate([A, Bm, A + Bm], axis=1), dtype=np.float16)

    def wo_prep(w_top, w_bot):
        Ct = w_top[:, hs:he].T.reshape(HPC, D, E)
        Dt = w_bot[:, hs:he].T.reshape(HPC, D, E)
        return np.concatenate([Ct, Dt], axis=1).reshape(2 * EH, E)

    # wot[p, part, m, j, n] = wo_part[j*128 + p, m*128 + n]
    wor = wo_prep(inp["wo_r"], -inp["wo_i"])
    woi = wo_prep(inp["wo_i"], inp["wo_r"])
    wo_st = np.stack([wor, woi])                      # [2, 1024, 1024]
    wo_st = wo_st.reshape(2, 8, 128, 8, 128)          # [part, j, p, m, n]
    wot = np.ascontiguousarray(
        wo_st.transpose(2, 0, 3, 1, 4).reshape(128, 16 * 8 * 128),
        dtype=np.float16)

    bneg = np.empty((128, 4), np.float32)
    bdel = np.empty((128, 4), np.float32)
    for rt in range(4):
        for par in range(2):
            h = hg * HPC + 2 * rt + par
            sl = slice(par * 64, (par + 1) * 64)
            br = inp["bq_r"][h * D:(h + 1) * D]
            bi = inp["bq_i"][h * D:(h + 1) * D]
            bneg[sl, rt] = -br
            bdel[sl, rt] = bi - br

    return dict(
        xq=xcat2(inp["query_r"], inp["query_i"]),
        xk=xcat2(inp["key_r"], inp["key_i"]),
        xv=xcat2(inp["value_r"], inp["value_i"]),
        wq=wv_prep(inp["wq_r"], inp["wq_i"]),
        wk=wv_prep(inp["wk_r"], inp["wk_i"]),
        wv=wv_prep(inp["wv_r"], inp["wv_i"]),
        wot=wot,
        bneg=bneg,
        bdel=bdel,
    )


def host_combine(results, inp):
    """Sum per-core partials, add the host-side constant, untranspose."""
    bvr = inp["bv_r"].astype(np.float64)
    bvi = inp["bv_i"].astype(np.float64)
    wr = inp["wo_r"].astype(np.float64)
    wi = inp["wo_i"].astype(np.float64)
    vb_r = bvr - bvi
    vb_i = bvr + bvi
    yc_r = (wr @ vb_r - wi @ vb_i + inp["bo_r"]).astype(np.float32)
    yc_i = (wr @ vb_i + wi @ vb_r + inp["bo_i"]).astype(np.float32)

    out = np.empty((S, B, E, 2), np.float32)
    for b in range(B):
        yr = (results[2 * b]["ytr"].astype(np.float32)
              + results[2 * b + 1]["ytr"].astype(np.float32))
        yi = (results[2 * b]["yti"].astype(np.float32)
              + results[2 * b + 1]["yti"].astype(np.float32))
        out[:, b, :, 0] = yr.T + yc_r
        out[:, b, :, 1] = yi.T + yc_i
    return out


def kernel(**inputs):
    inputs = {k: np.asarray(v) for k, v in inputs.items()}
    nc = _get_nc()
    in_maps = [prep_core(inputs, c) for c in range(N_CORES)]
    res = run_bass_kernel_spmd(nc, in_maps, core_ids=list(range(N_CORES)))
    return host_combine(res.results, inputs)


# revision 39
# speedup vs baseline: 1.1804x; 1.0118x over previous
"""Complex multihead attention v3: fp16 PE, V-Karatsuba, transposed AV with
fused softmax sums, DMA transposes, qi-outer loop with inline O projection.

Sharding: data-parallel over batch (B=4) x tensor-parallel over heads
(16 heads -> 2 groups of 8). core = b*2 + head_group. Host combines partials.

Math notes (validated against reference):
 - K bias dropped (softmax invariant); V bias folded to host constant;
   Q bias applied at PSUM evacuation (per-partition ACT bias).
 - Q projection: stacked-real trick -> qcat = [qr; qi] per head.
 - K projection: same stacked psum [kr; ki], evacuated TWICE with
   cross-partition-offset activations: kcat1 = [kr; -ki], kcat2 = [ki; kr],
   so scores are st_r = kcat1^T qcat, st_i = kcat2^T qcat (no per-head
   query prep in the attention phase).
 - Q/K/V projections: Karatsuba M1=xr*A, M2=xi*B, M3=(xr+xi)*(A+B) with
   xs = xr+xi computed on-device (DVE) to cut x DMA traffic by 1/3.
 - Scores transposed [k, q]; exp on scalar engine (fp16 out).
 - AV transposed: out[q, f] = P[:,qc]^T @ [1|Vr|Vi]: softmax sums ride in
   col 0; normalizer = per-partition activation scale. Both r and i P parts
   multiply the SAME [1|Vr|Vi] tile (no [1|Vi|Vr] mirror): bankR = [s|PrVr|
   PrVi], bankI = [s|PiVr|PiVi]; combine crosses halves. attn [q,f] -> [f,q]
   via DMA xbar transpose (split in partition halves for latency).
 - O projection per qi-half inline (overlaps the other half's attention).
 - DMAs: consumption-ordered, section/half granularity; 40-buf x pool so
   next-phase x prefetch streams behind the current phase's matmuls.
"""

import numpy as np

import concourse.bass as bass
from concourse import bacc
import concourse.mybir as mybir
import concourse.tile as tile
from concourse.bass_utils import run_bass_kernel_spmd

S, B, E, H, D = 1024, 4, 1024, 16, 64
HPC = 8            # heads per core
EH = HPC * D       # 512
N_CORES = 8
F32 = mybir.dt.float32
F16 = mybir.dt.float16
AF = mybir.ActivationFunctionType

_NC_CACHE = []

VW = 130           # vext per-head stride ([1|Vr|Vi] = 129, +1 pad)


def _emit(tc):
    nc = tc.nc
    # x payload: rows 0:1024 = x_r^T, rows 1024:2048 = x_i^T (xs on-device)
    xq = nc.dram_tensor("xq", [2 * E, S], F16, kind="ExternalInput").ap()
    xk = nc.dram_tensor("xk", [2 * E, S], F16, kind="ExternalInput").ap()
    xv = nc.dram_tensor("xv", [2 * E, S], F16, kind="ExternalInput").ap()
    wq = nc.dram_tensor("wq", [E, 3 * EH], F16, kind="ExternalInput").ap()
    wk = nc.dram_tensor("wk", [E, 3 * EH], F16, kind="ExternalInput").ap()
    wv = nc.dram_tensor("wv", [E, 3 * EH], F16, kind="ExternalInput").ap()
    # O-proj weights pre-swizzled on host: [128, (part,m,j,n)] so each
    # (part,m) chunk is one contiguous 2KB-per-partition DMA.
    wot = nc.dram_tensor("wot", [128, 16 * 8 * 128], F16,
                         kind="ExternalInput").ap()
    # col rt: lanes 0:64 = -bq_r(head 2rt), 64:128 = -bq_r(head 2rt+1)
    bneg = nc.dram_tensor("bneg", [128, 4], F32, kind="ExternalInput").ap()
    # col rt: (bq_i - bq_r) in the same lane layout
    bdel = nc.dram_tensor("bdel", [128, 4], F32, kind="ExternalInput").ap()
    ytr = nc.dram_tensor("ytr", [E, S], F16, kind="ExternalOutput").ap()
    yti = nc.dram_tensor("yti", [E, S], F16, kind="ExternalOutput").ap()

    store = tc.alloc_tile_pool(name="store", bufs=1)
    qcat = store.tile([128, HPC * S], F16)    # per head j: [qr; qi]
    kcat1 = store.tile([128, HPC * S], F16)   # per head j: [kr; -ki]
    kcat2 = store.tile([128, HPC * S], F16)   # per head j: [ki; kr]
    vext = store.tile([128, 8 * HPC * VW], F16)
    bneg_sb = store.tile([128, 4], F32)
    bdel_sb = store.tile([128, 4], F32)
    nc.sync.dma_start(out=bneg_sb, in_=bneg)
    nc.sync.dma_start(out=bdel_sb, in_=bdel)

    def vr_view(st):  # [128, 8 heads, VW] view of vext for st-tile
        return vext.rearrange("p (t j w) -> p t j w", t=8, j=HPC, w=VW)[:, st]

    # ---------------- Q/K/V projections (one pool scope) --------------------
    with tc.tile_pool(name="xp", bufs=40) as xp, \
         tc.tile_pool(name="wp", bufs=16) as wp, \
         tc.tile_pool(name="sc", bufs=2) as sc, \
         tc.tile_pool(name="pp", bufs=8, space="PSUM") as pp:

        def phase_dmas(which, xdram, wdram, fine):
            """Emit this phase's DMAs + xs adds in consumption order.

            DMA-instruction issue costs ~600ns serially per HWDGE engine,
            so instructions alternate between the SP and Activation HWDGEs
            (2x issue rate) and stay coarse.  `fine` (Q phase) orders
            w-section0 / h0 halves first for a fast pipeline lead-in;
            other phases are prefetched far ahead and use whole tiles.
            """
            eng = (nc.sync, nc.scalar)
            wts = [wp.tile([128, 3 * EH], F16, tag="w", name=f"w{which}{k}")
                   for k in range(8)]
            xr = [xp.tile([128, S], F16, tag="x", name=f"{which}xr{k}")
                  for k in range(8)]
            xi = [xp.tile([128, S], F16, tag="x", name=f"{which}xi{k}")
                  for k in range(8)]
            xs = [xp.tile([128, S], F16, tag="x", name=f"{which}xs{k}")
                  for k in range(8)]
            if fine:
                for k in range(8):
                    r0 = k * 128
                    eng[k % 2].dma_start(out=wts[k][:, 0:512],
                                         in_=wdram[r0:r0 + 128, 0:512])
                # interleave xr/xi per k so the xs DVE adds (m3 feed) can
                # fire as early as possible during the warmup ramp
                for k in range(8):
                    r0 = k * 128
                    eng[k % 2].dma_start(out=xr[k][:, 0:512],
                                         in_=xdram[r0:r0 + 128, 0:512])
                    eng[(k + 1) % 2].dma_start(
                        out=xi[k][:, 0:512],
                        in_=xdram[E + r0:E + r0 + 128, 0:512])
                for k in range(8):
                    nc.vector.tensor_add(xs[k][:, 0:512], xr[k][:, 0:512],
                                         xi[k][:, 0:512])
                for s in (1, 2):
                    for k in range(8):
                        r0 = k * 128
                        eng[k % 2].dma_start(
                            out=wts[k][:, s * 512:(s + 1) * 512],
                            in_=wdram[r0:r0 + 128, s * 512:(s + 1) * 512])
                for k in range(8):
                    r0 = k * 128
                    eng[k % 2].dma_start(out=xr[k][:, 512:1024],
                                         in_=xdram[r0:r0 + 128, 512:1024])
                for k in range(8):
                    r0 = E + k * 128
                    eng[k % 2].dma_start(out=xi[k][:, 512:1024],
                                         in_=xdram[r0:r0 + 128, 512:1024])
                for k in range(8):
                    nc.vector.tensor_add(xs[k][:, 512:1024],
                                         xr[k][:, 512:1024],
                                         xi[k][:, 512:1024])
            else:
                # sync-only: a scalar-issued DMA emitted after this phase's
                # evacuation activations would park in the scalar queue and
                # stall them (in-order engine queues).
                for k in range(8):
                    r0 = k * 128
                    nc.sync.dma_start(out=wts[k],
                                      in_=wdram[r0:r0 + 128, :])
                for k in range(8):
                    r0 = k * 128
                    nc.sync.dma_start(out=xr[k],
                                      in_=xdram[r0:r0 + 128, :])
                for k in range(8):
                    r0 = E + k * 128
                    nc.sync.dma_start(out=xi[k],
                                      in_=xdram[r0:r0 + 128, :])
                for k in range(8):
                    nc.vector.tensor_add(xs[k], xr[k], xi[k])
            return wts, xr, xi, xs

        def qk_matmuls(which, wts, xr, xi, xs):
            # hf (sequence half) OUTER so h1 x data has a full half-phase
            # of matmul time to stream in behind the PE.
            for hf in range(2):
                rx = slice(hf * 512, (hf + 1) * 512)
                for rt in range(4):
                    je, jo = 2 * rt, 2 * rt + 1
                    c = rt * 128
                    m1 = pp.tile([128, 512], F32, tag="pp", bufs=8,
                                 name=f"{which}m1_{rt}{hf}")
                    m2 = pp.tile([128, 512], F32, tag="pp", bufs=8,
                                 name=f"{which}m2_{rt}{hf}")
                    m3 = pp.tile([128, 512], F32, tag="pp", bufs=8,
                                 name=f"{which}m3_{rt}{hf}")
                    for k in range(8):
                        nc.tensor.matmul(m1, wts[k][:, c:c + 128],
                                         xr[k][:, rx],
                                         start=(k == 0), stop=(k == 7))
                    for k in range(8):
                        nc.tensor.matmul(m2, wts[k][:, 512 + c:512 + c + 128],
                                         xi[k][:, rx],
                                         start=(k == 0), stop=(k == 7))
                    for k in range(8):
                        nc.tensor.matmul(m3, wts[k][:, 1024 + c:1024 + c + 128],
                                         xs[k][:, rx],
                                         start=(k == 0), stop=(k == 7))
                    m2s = sc.tile([128, 512], F32, tag="m2s", bufs=2,
                                  name=f"{which}m2s{rt}{hf}")
                    tms = sc.tile([128, 512], F32, tag="tms", bufs=2,
                                  name=f"{which}tms{rt}{hf}")
                    if which == "q":
                        nc.scalar.activation(m2s, m2, AF.Identity,
                                             bias=bneg_sb[:, rt:rt + 1])
                        nc.scalar.activation(tms, m3, AF.Identity,
                                             bias=bdel_sb[:, rt:rt + 1])
                    else:
                        nc.scalar.activation(m2s, m2, AF.Copy)
                        nc.scalar.activation(tms, m3, AF.Copy)
                    sv = sc.tile([128, 512], F32, tag="sv", bufs=2,
                                 name=f"{which}sv{rt}{hf}")
                    nc.vector.tensor_sub(sv, tms, m2s)   # M3 - M2 (+bias)
                    c0e = je * S + hf * 512
                    c0o = jo * S + hf * 512
                    if which == "q":
                        # qcat[j] = [qr; qi]
                        nc.vector.tensor_sub(qcat[0:64, c0e:c0e + 512],
                                             m1[0:64, :], m2s[0:64, :])
                        nc.vector.tensor_sub(qcat[0:64, c0o:c0o + 512],
                                             m1[64:128, :], m2s[64:128, :])
                        nc.vector.tensor_sub(qcat[64:128, c0e:c0e + 512],
                                             sv[0:64, :], m1[0:64, :])
                        nc.vector.tensor_sub(qcat[64:128, c0o:c0o + 512],
                                             sv[64:128, :], m1[64:128, :])
                    else:
                        # kcat1[j] = [kr; -ki], kcat2[j] = [ki; kr]
                        nc.vector.tensor_sub(kcat1[0:64, c0e:c0e + 512],
                                             m1[0:64, :], m2s[0:64, :])
                        nc.vector.tensor_sub(kcat1[0:64, c0o:c0o + 512],
                                             m1[64:128, :], m2s[64:128, :])
                        nc.vector.tensor_sub(kcat1[64:128, c0e:c0e + 512],
                                             m1[0:64, :], sv[0:64, :])
                        nc.vector.tensor_sub(kcat1[64:128, c0o:c0o + 512],
                                             m1[64:128, :], sv[64:128, :])
                        nc.vector.tensor_sub(kcat2[0:64, c0e:c0e + 512],
                                             sv[0:64, :], m1[0:64, :])
                        nc.vector.tensor_sub(kcat2[0:64, c0o:c0o + 512],
                                             sv[64:128, :], m1[64:128, :])
                        nc.vector.tensor_copy(kcat2[64:128, c0e:c0e + 512],
                                              kcat1[0:64, c0e:c0e + 512])
                        nc.vector.tensor_copy(kcat2[64:128, c0o:c0o + 512],
                                              kcat1[0:64, c0o:c0o + 512])

        # Q phase (fine lead-in pieces), then K, then V; each phase's DMAs
        # are emitted before its matmuls, so the SP DMA queue streams the
        # next phase's data while the PE chews on the current one.
        wtq, xrq, xiq, xsq = phase_dmas("q", xq, wq, fine=True)
        qk_matmuls("q", wtq, xrq, xiq, xsq)
        wtk, xrk, xik, xsk = phase_dmas("k", xk, wk, fine=False)
        qk_matmuls("k", wtk, xrk, xik, xsk)
        wtv, xrv, xiv, xsv = phase_dmas("v", xv, wv, fine=False)

        # ------------- V projection (Karatsuba, natural layout) -------------
        for st in range(8):
            m1 = pp.tile([128, 512], F32, tag="pp", bufs=8, name=f"vm1_{st}")
            m2 = pp.tile([128, 512], F32, tag="pp", bufs=8, name=f"vm2_{st}")
            m3 = pp.tile([128, 512], F32, tag="pp", bufs=8, name=f"vm3_{st}")
            cs = slice(st * 128, (st + 1) * 128)
            for k in range(8):
                nc.tensor.matmul(m1, xrv[k][:, cs], wtv[k][:, 0:512],
                                 start=(k == 0), stop=(k == 7))
            for k in range(8):
                nc.tensor.matmul(m2, xiv[k][:, cs], wtv[k][:, 512:1024],
                                 start=(k == 0), stop=(k == 7))
            for k in range(8):
                nc.tensor.matmul(m3, xsv[k][:, cs], wtv[k][:, 1024:1536],
                                 start=(k == 0), stop=(k == 7))
            # Vr = m1 - m2, Vi = m3 - m1 - m2
            m2s = sc.tile([128, 512], F32, tag="m2s", bufs=2, name=f"m2s{st}")
            nc.scalar.activation(m2s, m2, AF.Copy)
            sv = sc.tile([128, 512], F32, tag="sv", bufs=2, name=f"sv{st}")
            nc.vector.tensor_sub(sv, m3, m2s)          # m3 - m2
            vr = vr_view(st)
            m1v = m1.rearrange("p (j d) -> p j d", j=HPC)
            svv = sv.rearrange("p (j d) -> p j d", j=HPC)
            m2v = m2s.rearrange("p (j d) -> p j d", j=HPC)
            nc.vector.tensor_sub(vr[:, :, 1:65], m1v, m2v)
            nc.vector.tensor_sub(vr[:, :, 65:129], svv, m1v)
            nc.vector.memset(vr[:, :, 0:1], 1.0)

    # ---------------- O-projection weight prefetch --------------------------
    # (fresh pool; DMAs drain during late projections / early attention)
    wop = tc.alloc_tile_pool(name="wop", bufs=1)
    wo_sb = wop.tile([128, 16, 8, 128], F16)
    for i in range(16):
        nc.sync.dma_start(out=wo_sb[:, i],
                          in_=wot[:, i * 1024:(i + 1) * 1024])
    afp = tc.alloc_tile_pool(name="afp", bufs=1)
    attn_fs = afp.tile([128, HPC * S], F16)  # per head j: [or_d; oi_d] x q

    # ---------------- attention (qi outer) + inline O projection ------------
    with tc.tile_pool(name="asb", bufs=2) as asb, \
         tc.tile_pool(name="stp", bufs=2, space="PSUM") as stp, \
         tc.tile_pool(name="avp", bufs=4, space="PSUM") as avp, \
         tc.tile_pool(name="ytp", bufs=4) as ytp:

        def emit_oproj_piece(qi, part, m, last=False):
            sq0 = qi * 512
            yt_d = ytr if part == 0 else yti
            wt = wo_sb[:, part * 8 + m]
            pso = avp.tile([128, 512], F32, tag="av", name=f"po{part}{m}{qi}")
            for jj in range(HPC):
                nc.tensor.matmul(
                    pso, wt[:, jj, :],
                    attn_fs[:, jj * S + sq0: jj * S + sq0 + 512],
                    start=(jj == 0), stop=(jj == 7))
            yt_t = ytp.tile([128, 512], F16, tag="yt", name=f"yt{part}{m}{qi}")
            nc.vector.tensor_copy(yt_t, pso)
            if last:
                # split across both HWDGEs for tail latency (the scalar
                # exp stream is already done at this point)
                nc.sync.dma_start(
                    out=yt_d[m * 128:(m + 1) * 128, sq0:sq0 + 256],
                    in_=yt_t[:, 0:256])
                nc.scalar.dma_start(
                    out=yt_d[m * 128:(m + 1) * 128, sq0 + 256:sq0 + 512],
                    in_=yt_t[:, 256:512])
            else:
                nc.sync.dma_start(
                    out=yt_d[m * 128:(m + 1) * 128, sq0:sq0 + 512],
                    in_=yt_t)

        def oslice(bks, ri, qc):
            bankR, bankI, bankM = bks
            if qc < 3:
                b = bankR if ri == 0 else bankI
                return b[:, qc * 129:qc * 129 + 129]
            return bankM[:, ri * 129:ri * 129 + 129]

        def emit_scores(qi, j, t):
            sq0 = qi * 512
            qh = qcat[:, j * S + sq0: j * S + sq0 + 512]
            # one 2-bank tile: r scores in [:,0:512], i in [:,512:]
            st_t = stp.tile([128, 1024], F32, tag="st", name=f"st{j}{qi}{t}")
            kc = j * S + t * 128
            nc.tensor.matmul(st_t[:, 0:512], kcat1[:, kc:kc + 128],
                             qh, start=True, stop=True)
            nc.tensor.matmul(st_t[:, 512:1024], kcat2[:, kc:kc + 128],
                             qh, start=True, stop=True)
            pt = asb.tile([128, 1024], F16, tag="pt", bufs=3,
                          name=f"pt{j}{qi}{t}")
            nc.scalar.activation(pt, st_t, AF.Exp, scale=0.125)
            return pt

        def emit_av(qi, j, t, pt, bks):
            vrt = vr_view(t)[:, j, 0:129]
            # start=True zeroes the whole 2KB bank: only the first
            # group per bank starts; only the last group stops.
            for qc in range(4):
                nc.tensor.matmul(oslice(bks, 0, qc),
                                 pt[:, qc * 128:(qc + 1) * 128], vrt,
                                 start=(t == 0 and qc in (0, 3)),
                                 stop=(t == 7 and qc == 2))
                nc.tensor.matmul(oslice(bks, 1, qc),
                                 pt[:, 512 + qc * 128: 512 + (qc + 1) * 128],
                                 vrt,
                                 start=(t == 0 and qc == 0),
                                 stop=(t == 7 and qc in (2, 3)))

        def emit_finalize(qi, j, bks):
            # finalize: rcp (DVE), normalize on DVE r-part first (frees the
            # PSUM banks for the next head's AV in WAR-chain order),
            # combine (gpsimd), transpose (DMA xbar on the Sync engine).
            sq0 = qi * 512
            bankR, bankI, bankM = bks
            rcp = asb.tile([128, 2, 4], F32, tag="rcp", name=f"rcp{j}{qi}")
            bRv = bankR[:, 0:387].rearrange("p (g w) -> p g w", w=129, g=3)
            bIv = bankI[:, 0:387].rearrange("p (g w) -> p g w", w=129, g=3)
            bMv = bankM[:, 0:258].rearrange("p (g w) -> p g w", w=129, g=2)
            nc.vector.reciprocal(rcp[:, 0, 0:3], bRv[:, :, 0])
            nc.vector.reciprocal(rcp[:, 1, 0:3], bIv[:, :, 0])
            nc.vector.reciprocal(rcp[:, :, 3], bMv[:, :, 0])
            tmp_r = asb.tile([128, 4, 128], F16, tag="tmr", name=f"tr{j}{qi}")
            tmp_i = asb.tile([128, 4, 128], F16, tag="tmi", name=f"ti{j}{qi}")
            for qc in range(4):
                nc.vector.tensor_scalar_mul(
                    tmp_r[:, qc, :], oslice(bks, 0, qc)[:, 1:129],
                    rcp[:, 0, qc:qc + 1])
            for qc in range(4):
                nc.vector.tensor_scalar_mul(
                    tmp_i[:, qc, :], oslice(bks, 1, qc)[:, 1:129],
                    rcp[:, 1, qc:qc + 1])
            # tmp_r = [PrVr | PrVi], tmp_i = [PiVr | PiVi] (both from the
            # same [1|Vr|Vi] tile): real = PrVr - PiVi crosses halves.
            attn_sb = asb.tile([128, 4, 128], F16, tag="ats", name=f"as{j}{qi}")
            nc.gpsimd.tensor_sub(attn_sb[:, :, 0:64], tmp_r[:, :, 0:64],
                                 tmp_i[:, :, 64:128])
            nc.gpsimd.tensor_add(attn_sb[:, :, 64:128], tmp_r[:, :, 64:128],
                                 tmp_i[:, :, 0:64])
            # DMA transposes run ~1.2us each on the issuing ENGINE, so keep
            # them whole; the last head's four (tail critical path) split
            # across the two HWDGE engines (exp stream is done by then).
            last = (qi == 1 and j == HPC - 1)
            for qc in range(4):
                c0 = j * S + sq0 + qc * 128
                e = nc.scalar if (last and qc % 2) else nc.sync
                e.dma_start_transpose(
                    attn_fs[:, c0:c0 + 128], attn_sb[:, qc, :])

        # Per-head software pipeline: scores/exp one t-stage ahead of AV so
        # the in-order engine queues keep the PE busy during exp.
        for qi in range(2):
            for j in range(HPC):
                bks = (avp.tile([128, 512], F32, tag="av", name=f"bR{j}{qi}"),
                       avp.tile([128, 512], F32, tag="av", name=f"bI{j}{qi}"),
                       avp.tile([128, 512], F32, tag="av", name=f"bM{j}{qi}"))
                pts = {}
                for t in range(9):
                    if t < 8:
                        pts[t] = emit_scores(qi, j, t)
                    if t == 0:
                        continue
                    emit_av(qi, j, t - 1, pts.pop(t - 1), bks)
                emit_finalize(qi, j, bks)
                if qi == 1:
                    # interleave qi=0's O projection with qi=1's attention so
                    # O-proj matmuls fill the PE while exp runs on scalar.
                    for p in (2 * j, 2 * j + 1):
                        emit_oproj_piece(0, p // 8, p % 8)

        # tail: O projection for qi=1
        for part in range(2):
            for m in range(8):
                emit_oproj_piece(1, part, m, last=(part == 1 and m >= 6))

    afp.release()
    wop.release()
    store.release()


def build_module():
    nc = bacc.Bacc("TRN2", target_bir_lowering=False)
    with tile.TileContext(nc) as tc:
        _emit(tc)
    nc.compile()
    return nc


def _get_nc():
    if not _NC_CACHE:
        _NC_CACHE.append(build_module())
    return _NC_CACHE[0]


def prep_core(inp, core):
    """Host-side shard prep for one core."""
    b, hg = divmod(core, 2)
    hs, he = hg * EH, (hg + 1) * EH

    def xcat2(xr, xi):
        a = xr[:, b, :].T.astype(np.float16)
        c = xi[:, b, :].T.astype(np.float16)
        return np.ascontiguousarray(np.concatenate([a, c], axis=0))

    def wv_prep(wr, wi):
        A = wr[hs:he, :].T.astype(np.float32)
        Bm = wi[hs:he, :].T.astype(np.float32)
        return np.ascontiguousarray(
            np.concatenate([A, Bm, A + Bm], axis=1), dtype=np.float16)

    def wo_prep(w_top, w_bot):
        Ct = w_top[:, hs:he].T.reshape(HPC, D, E)
        Dt = w_bot[:, hs:he].T.reshape(HPC, D, E)
        return np.concatenate([Ct, Dt], axis=1).reshape(2 * EH, E)

    # wot[p, part, m, j, n] = wo_part[j*128 + p, m*128 + n]
    wor = wo_prep(inp["wo_r"], -inp["wo_i"])
    woi = wo_prep(inp["wo_i"], inp["wo_r"])
    wo_st = np.stack([wor, woi])                      # [2, 1024, 1024]
    wo_st = wo_st.reshape(2, 8, 128, 8, 128)          # [part, j, p, m, n]
    wot = np.ascontiguousarray(
        wo_st.transpose(2, 0, 3, 1, 4).reshape(128, 16 * 8 * 128),
        dtype=np.float16)

    bneg = np.empty((128, 4), np.float32)
    bdel = np.empty((128, 4), np.float32)
    for rt in range(4):
        for par in range(2):
            h = hg * HPC + 2 * rt + par
            sl = slice(par * 64, (par + 1) * 64)
            br = inp["bq_r"][h * D:(h + 1) * D]
            bi = inp["bq_i"][h * D:(h + 1) * D]
            bneg[sl, rt] = -br
            bdel[sl, rt] = bi - br

    return dict(
        xq=xcat2(inp["query_r"], inp["query_i"]),
        xk=xcat2(inp["key_r"], inp["key_i"]),
        xv=xcat2(inp["value_r"], inp["value_i"]),
        wq=wv_prep(inp["wq_r"], inp["wq_i"]),
        wk=wv_prep(inp["wk_r"], inp["wk_i"]),
        wv=wv_prep(inp["wv_r"], inp["wv_i"]),
        wot=wot,
        bneg=bneg,
        bdel=bdel,
    )


def host_combine(results, inp):
    """Sum per-core partials, add the host-side constant, untranspose."""
    bvr = inp["bv_r"].astype(np.float64)
    bvi = inp["bv_i"].astype(np.float64)
    wr = inp["wo_r"].astype(np.float64)
    wi = inp["wo_i"].astype(np.float64)
    vb_r = bvr - bvi
    vb_i = bvr + bvi
    yc_r = (wr @ vb_r - wi @ vb_i + inp["bo_r"]).astype(np.float32)
    yc_i = (wr @ vb_i + wi @ vb_r + inp["bo_i"]).astype(np.float32)

    out = np.empty((S, B, E, 2), np.float32)
    for b in range(B):
        yr = (results[2 * b]["ytr"].astype(np.float32)
              + results[2 * b + 1]["ytr"].astype(np.float32))
        yi = (results[2 * b]["yti"].astype(np.float32)
              + results[2 * b + 1]["yti"].astype(np.float32))
        out[:, b, :, 0] = yr.T + yc_r
        out[:, b, :, 1] = yi.T + yc_i
    return out


def kernel(**inputs):
    inputs = {k: np.asarray(v) for k, v in inputs.items()}
    nc = _get_nc()
    in_maps = [prep_core(inputs, c) for c in range(N_CORES)]
    res = run_bass_kernel_spmd(nc, in_maps, core_ids=list(range(N_CORES)))
    return host_combine(res.results, inputs)
